# revision 24
# baseline (speedup 1.0000x reference)
"""Trainium2 Bass kernel for nn_ExecPolicyNetwork (ragged repeat + 3-layer MLP).

Math (reference):
    x_dag = x[ptr[:-1], :16][job_indices]                       # [N, 16]
    u = [x_dag | h_dag]  (80)   v = h_glob (64)
    dag_inputs[t] = [u[job(t)] | v[job(t)] | a_t]               # [T, 145]
    out = tanh(tanh(dag_inputs @ W1 + b1) @ W2 + b2) @ W3 + b3  # [T]

Kernel factorization: layer 1 is computed per-JOB (projUV = [u|v] @ W1[:144],
20000 rows instead of 1,010,000), then expanded to the ragged action dim by a
one-hot matmul on the PE that simultaneously adds the a_t * W1[144] rank-1 term:
    h1_pre[:, t] = [projUV_rows; w_a]^T @ [onehot(job)_cols; a_row]
Sharding: data-parallel over jobs, 8 contiguous slices of 2500 jobs (each
exactly 25 cycles of k=1..100 -> identical ragged structure on every core, so
one SPMD program serves all cores). All compute in fp16 (exact 0/1 one-hots,
fp32 PSUM accumulate); measured end-to-end relative error ~6e-4.

Per core, actions are processed in groups of 2048 (4 matmul tiles of 512).
PSUM (8 banks): exp ring [128,2048] banks 0-3, L2 ring [128,1024] banks 4-5,
L3+phase-1 in banks 6-7. Layer 2 runs as col-tiled concurrent pairs
(tile_position (0,0)/(0,64)); layer 3 as col-tiled concurrent pairs into one
bank (rows 0-1 / 32-33). Emission is software-pipelined: L3(g-1) fills the
PE's tanh1(g) wait window.
"""

import os
import numpy as np
from contextlib import ExitStack

from concourse import bacc, tile, mybir
from concourse.bass_utils import run_bass_kernel_spmd
from concourse._compat import with_exitstack

F32 = mybir.dt.float32
F16 = mybir.dt.float16
Tanh = mybir.ActivationFunctionType.Tanh
ADD = mybir.AluOpType.add

N_CORES = 8
NUM_DAG_FEATURES = 16
TILE = 512
TILES_PER_GROUP = 4
GROUP = TILE * TILES_PER_GROUP  # 2048

_cache = {}
last_results = None


def _f16(a):
    return np.ascontiguousarray(a, dtype=np.float16)


def _ensure_ntff_hook():
    """This image lacks antenv.axon_hooks; synthesize it so trace=True can
    capture NTFF profiles via /opt/axon/libaxon_pjrt.so."""
    import sys, types, ctypes, contextlib
    try:
        from antenv.axon_hooks import get_axon_ntff_profile_hook  # noqa: F401
        return
    except ImportError:
        pass
    so_path = "/opt/axon/libaxon_pjrt.so"
    if not os.path.exists(so_path):
        return
    lib = ctypes.CDLL(so_path)
    if not hasattr(lib, "axon_start_nrt_profile"):
        return
    lib.axon_start_nrt_profile.argtypes = [ctypes.POINTER(ctypes.c_int64), ctypes.c_size_t]
    lib.axon_start_nrt_profile.restype = ctypes.c_int64
    lib.axon_stop_nrt_profile.argtypes = [ctypes.c_char_p]
    lib.axon_stop_nrt_profile.restype = ctypes.c_int64

    @contextlib.contextmanager
    def _hook(output_dir, device_ids):
        import jax
        jax.devices()
        if device_ids:
            ids = (ctypes.c_int64 * len(device_ids))(*device_ids)
            rc = lib.axon_start_nrt_profile(ids, len(device_ids))
        else:
            rc = lib.axon_start_nrt_profile(None, 0)
        if rc != 0:
            raise RuntimeError(f"axon_start_nrt_profile rc={rc}")
        try:
            yield
        finally:
            n = lib.axon_stop_nrt_profile(str(output_dir).encode())
            print(f"ntff profile: {n} file(s) -> {output_dir}", file=sys.stderr)

    mod = types.ModuleType("antenv.axon_hooks")
    mod._hook = _hook
    mod.get_axon_ntff_profile_hook = lambda: _hook
    mod.set_axon_ntff_profile_hook = lambda h: setattr(mod, "_hook", h)
    import antenv
    sys.modules["antenv.axon_hooks"] = mod
    antenv.axon_hooks = mod


def _plan_core(k):
    """Static ragged plan for one core from its per-job action counts."""
    k = np.asarray(k, dtype=np.int64)
    nj = len(k)
    T = int(k.sum())
    n_groups = (T + GROUP - 1) // GROUP
    T_pad = n_groups * GROUP
    job_of_action = np.repeat(np.arange(nj), k)  # [T]
    start_of_job = np.concatenate([[0], np.cumsum(k)[:-1]])
    e_of_action = np.arange(T) - start_of_job[job_of_action]

    groups = []  # (jlo, jhi, srow_off, wide)
    srow_off = 0
    for g in range(n_groups):
        lo, hi = GROUP * g, min(GROUP * (g + 1), T)
        jlo = int(job_of_action[lo])
        jhi = int(job_of_action[hi - 1]) + 1
        assert jhi - jlo <= 67, "group spans too many jobs for one [68,128] lhsT"
        wide = (jhi - jlo + 1) > 64  # dual-copy lhsT needs rows at base 64
        groups.append((jlo, jhi, srow_off, wide))
        srow_off += (jhi - jlo) + 1
    return dict(
        T=T, T_pad=T_pad, n_groups=n_groups, groups=tuple(groups),
        srows=srow_off, job_of_action=job_of_action, e_of_action=e_of_action,
    )


def _build_spack(plan, num_exec):
    """Pack per-group [J_g+1, 2048] one-hot + a-row blocks into one array."""
    sp = np.zeros((plan["srows"], GROUP), dtype=np.float32)
    T = plan["T"]
    joa, eoa = plan["job_of_action"], plan["e_of_action"]
    a_vals = eoa.astype(np.float32) / np.float32(num_exec)
    for g, (jlo, jhi, off, wide) in enumerate(plan["groups"]):
        lo, hi = GROUP * g, min(GROUP * (g + 1), T)
        cols = np.arange(lo, hi) - lo
        sp[off + (joa[lo:hi] - jlo), cols] = 1.0
        sp[off + (jhi - jlo), cols] = a_vals[lo:hi]
    return _f16(sp)


@with_exitstack
def _emit(ctx: ExitStack, tc: tile.TileContext, io, plan):
    nc = tc.nc
    n_groups = plan["n_groups"]
    groups = plan["groups"]
    NJ = io["ut"].shape[1]

    pool = ctx.enter_context(tc.tile_pool(name="consts", bufs=1))
    s_pool = ctx.enter_context(tc.tile_pool(name="s", bufs=4))
    gw_pool = ctx.enter_context(tc.tile_pool(name="gw", bufs=4))
    h1_pool = ctx.enter_context(tc.tile_pool(name="h1", bufs=2))
    h2_pool = ctx.enter_context(tc.tile_pool(name="h2", bufs=2))
    st_pool = ctx.enter_context(tc.tile_pool(name="st", bufs=3))

    t_ut = pool.tile([80, NJ], F16, tag="ut")
    nc.sync.dma_start(t_ut[:], io["ut"][:])
    t_vt = pool.tile([64, NJ], F16, tag="vt")
    nc.sync.dma_start(t_vt[:], io["vt"][:])
    t_w1a = pool.tile([80, 128], F16, tag="w1a")
    nc.sync.dma_start(t_w1a[:], io["w1a"][:])
    t_w1b = pool.tile([64, 128], F16, tag="w1b")
    nc.sync.dma_start(t_w1b[:], io["w1b"][:])
    t_b1 = pool.tile([128, 1], F32, tag="b1")
    nc.sync.dma_start(t_b1[:], io["b1"][:])
    t_w2 = pool.tile([128, 64], F16, tag="w2")
    nc.sync.dma_start(t_w2[:], io["w2"][:])
    t_b22 = pool.tile([128, 1], F32, tag="b22")
    nc.sync.dma_start(t_b22[:], io["b22"][:])
    t_w3 = pool.tile([128, 2], F16, tag="w3blk")
    nc.sync.dma_start(t_w3[:], io["w3blk"][:])
    t_b34 = pool.tile([34, 1], F32, tag="b34")
    nc.sync.dma_start(t_b34[:], io["b34"][:])

    # PSUM: exp ring banks 0-3, L2 ring banks 4-5, L3 + phase-1 banks 6-7.
    EXPR = nc.alloc_psum_tensor("EXPR", [128, GROUP], F32)
    L2R = nc.alloc_psum_tensor("L2R", [128, 1024], F32)
    R3 = [nc.alloc_psum_tensor(f"R3{i}", [128, 512], F32) for i in range(2)]

    s_tiles, gw_tiles, h1_tiles, h2_tiles = {}, {}, {}, {}

    def emit_s_dma(g):
        if g >= n_groups:
            return
        jlo, jhi, off, wide = groups[g]
        rows = jhi - jlo + 1
        t = s_pool.tile([128, GROUP], F16, tag="s")
        nc.sync.dma_start(t[0:rows, :], io["spack"][off:off + rows, :])
        if not wide:
            # replicate to partition base 64 for the concurrent pair matmuls
            nc.gpsimd.dma_start(t[64:64 + rows, :], t[0:rows, :])
        s_tiles[g] = t

    def emit_ph1(g):
        """projUV for group g's jobs -> gw tile (dual copy at bases 0 and 64
        for narrow groups, written by col-tiled matmuls — concurrent)."""
        if g >= n_groups:
            return
        jlo, jhi, off, wide = groups[g]
        J = jhi - jlo
        c = 128 * (1 + (g // 2) % 3)
        pj0 = R3[g % 2].ap()[0:J, c:c + 128]
        nc.tensor.matmul(pj0, t_ut[:, jlo:jhi], t_w1a[:], start=True, stop=False,
                         tile_position=(0, 0))
        nc.tensor.matmul(pj0, t_vt[:, jlo:jhi], t_w1b[:], start=False, stop=True,
                         tile_position=(0, 0))
        t = gw_pool.tile([128, 128], F16, tag="gw")
        nc.vector.tensor_copy(t[0:J, :], pj0)
        # w_a row lands on partition J — DVE cannot shift partitions, DMA can
        nc.gpsimd.dma_start(t[J:J + 1, :], io["wa"][:])
        if not wide:
            pj1 = R3[g % 2].ap()[64:64 + J, c:c + 128]
            nc.tensor.matmul(pj1, t_ut[:, jlo:jhi], t_w1a[:], start=True,
                             stop=False, tile_position=(0, 64))
            nc.tensor.matmul(pj1, t_vt[:, jlo:jhi], t_w1b[:], start=False,
                             stop=True, tile_position=(0, 64))
            nc.vector.tensor_copy(t[64:64 + J, :], pj1)
            nc.gpsimd.dma_start(t[64 + J:64 + J + 1, :], io["wa"][:])
        gw_tiles[g] = t

    def emit_exp(g):
        """Expansion for group g: tile pairs run concurrently in disjoint PE
        row-groups (lhsT/rhs copies at partition bases 0 and 64)."""
        if g >= n_groups:
            return
        jlo, jhi, off, wide = groups[g]
        rows = jhi - jlo + 1
        gw = gw_tiles.pop(g)
        s = s_tiles.pop(g)
        for t in range(TILES_PER_GROUP):
            base = 0 if (wide or t % 2 == 0) else 64
            nc.tensor.matmul(
                EXPR.ap()[:, TILE * t: TILE * (t + 1)],
                gw[base:base + rows, :],
                s[base:base + rows, TILE * t: TILE * (t + 1)],
                start=True, stop=True,
            )

    def emit_tanh1(g):
        """Two [128,1024] tanh ops so exp(g+1) can ring-reuse banks early."""
        if g >= n_groups:
            return
        h1 = h1_pool.tile([128, GROUP], F16, tag="h1")
        for h in range(2):
            nc.scalar.activation(h1[:, 1024 * h: 1024 * (h + 1)],
                                 EXPR.ap()[:, 1024 * h: 1024 * (h + 1)],
                                 Tanh, bias=t_b1[:, 0:1])
        h1_tiles[g] = h1

    def emit_l2(g):
        if g >= n_groups:
            return
        h1 = h1_tiles.pop(g)
        for p in range(2):
            nc.tensor.matmul(
                L2R.ap()[0:64, TILE * p: TILE * (p + 1)],
                t_w2[:], h1[:, 1024 * p: 1024 * p + 512],
                start=True, stop=True, tile_position=(0, 0),
            )
            nc.tensor.matmul(
                L2R.ap()[64:128, TILE * p: TILE * (p + 1)],
                t_w2[:], h1[:, 1024 * p + 512: 1024 * p + 1024],
                start=True, stop=True, tile_position=(0, 64),
            )

    def emit_tanh2(g):
        if g >= n_groups:
            return
        h2 = h2_pool.tile([128, 1024], F16, tag="h2")
        nc.scalar.activation(h2[:], L2R.ap()[:, 0:1024], Tanh, bias=t_b22[:, 0:1])
        h2_tiles[g] = h2

    def emit_l3_out(g):
        """Two col-tiled concurrent L3 matmuls into one R3 bank (rows 0-1 and
        32-33), one [34,512] DVE copy+bias, two out DMAs."""
        if g >= n_groups:
            return
        h2 = h2_tiles.pop(g)
        r3 = R3[g % 2].ap()
        nc.tensor.matmul(r3[0:2, 0:512], t_w3[:], h2[:, 0:512],
                         start=True, stop=True, tile_position=(0, 0))
        nc.tensor.matmul(r3[32:34, 0:512], t_w3[:], h2[:, 512:1024],
                         start=True, stop=True, tile_position=(0, 32))
        st = st_pool.tile([34, 512], F32, tag="st")
        nc.vector.tensor_scalar(st[:], r3[0:34, 0:512], t_b34[:, 0:1], None, ADD)
        nc.gpsimd.dma_start(io["out"][4 * g: 4 * g + 2, :], st[0:2, :])
        nc.gpsimd.dma_start(io["out"][4 * g + 2: 4 * g + 4, :], st[32:34, :])

    # ---- software-pipelined emission ----
    # PE order per iter: L3(g-1) fills the tanh1(g) wait, then L2(g),
    # ph1(g+2), exp(g+1). L3 is delayed one group (h2 pool holds it).
    emit_ph1(0)
    emit_ph1(1)
    emit_s_dma(0)
    emit_s_dma(1)
    emit_s_dma(2)
    emit_exp(0)
    for g in range(n_groups):
        emit_s_dma(g + 3)
        emit_tanh1(g)
        if g >= 1:
            emit_l3_out(g - 1)
        emit_l2(g)
        emit_ph1(g + 2)
        emit_exp(g + 1)
        emit_tanh2(g)
    emit_l3_out(n_groups - 1)


def _build(plan, nj):
    nc = bacc.Bacc(trn_type="TRN2", target_bir_lowering=False, debug=False)
    io = {
        "ut": nc.dram_tensor("ut", [80, nj], F16, kind="ExternalInput").ap(),
        "vt": nc.dram_tensor("vt", [64, nj], F16, kind="ExternalInput").ap(),
        "w1a": nc.dram_tensor("w1a", [80, 128], F16, kind="ExternalInput").ap(),
        "w1b": nc.dram_tensor("w1b", [64, 128], F16, kind="ExternalInput").ap(),
        "wa": nc.dram_tensor("wa", [1, 128], F16, kind="ExternalInput").ap(),
        "b1": nc.dram_tensor("b1", [128, 1], F32, kind="ExternalInput").ap(),
        "w2": nc.dram_tensor("w2", [128, 64], F16, kind="ExternalInput").ap(),
        "b22": nc.dram_tensor("b22", [128, 1], F32, kind="ExternalInput").ap(),
        "w3blk": nc.dram_tensor("w3blk", [128, 2], F16, kind="ExternalInput").ap(),
        "b34": nc.dram_tensor("b34", [34, 1], F32, kind="ExternalInput").ap(),
        "spack": nc.dram_tensor("spack", [plan["srows"], GROUP], F16, kind="ExternalInput").ap(),
        "out": nc.dram_tensor("out", [plan["T_pad"] // TILE, TILE], F32, kind="ExternalOutput").ap(),
    }
    with tile.TileContext(nc) as tc:
        _emit(tc, io, plan)
    nc.compile()
    return nc


def kernel(x, h_dag, h_glob, W1, b1, W2, b2, W3, b3,
           ptr, job_indices, exec_mask, num_exec_acts, total_actions):
    global last_results
    x = np.asarray(x, dtype=np.float32)
    h_dag = np.asarray(h_dag, dtype=np.float32)
    h_glob = np.asarray(h_glob, dtype=np.float32)
    W1 = np.asarray(W1, dtype=np.float32)
    b1 = np.asarray(b1, dtype=np.float32)
    W2 = np.asarray(W2, dtype=np.float32)
    b2 = np.asarray(b2, dtype=np.float32)
    W3 = np.asarray(W3, dtype=np.float32)
    b3 = np.asarray(b3, dtype=np.float32)
    ptr = np.asarray(ptr, dtype=np.int64)
    job_indices = np.asarray(job_indices, dtype=np.int64)
    k = np.asarray(num_exec_acts, dtype=np.int64)
    num_exec = np.asarray(exec_mask).shape[1]

    nj_total = len(job_indices)
    assert nj_total % N_CORES == 0
    nj = nj_total // N_CORES

    # per-job gathered features (host-side layout/gather only; no arithmetic)
    x_dag = x[ptr[:-1][job_indices], :NUM_DAG_FEATURES]  # [N, 16]

    # per-core plans must be identical (one SPMD program for all cores)
    plans = [_plan_core(k[c * nj:(c + 1) * nj]) for c in range(N_CORES)]
    key0 = (plans[0]["n_groups"], plans[0]["srows"], plans[0]["groups"])
    for p in plans[1:]:
        assert (p["n_groups"], p["srows"], p["groups"]) == key0, \
            "per-core ragged structures differ; SPMD single-program assumption violated"
    plan = plans[0]

    cache_key = (nj, key0)
    if cache_key not in _cache:
        _cache[cache_key] = _build(plan, nj)
    nc = _cache[cache_key]

    w3blk = np.zeros((128, 2), dtype=np.float32)
    w3blk[:64, 0] = W3[:, 0]
    w3blk[64:, 1] = W3[:, 0]
    shared = {
        "w1a": _f16(W1[:80]),
        "w1b": _f16(W1[80:144]),
        "wa": _f16(W1[144:145]),
        "b1": np.ascontiguousarray(b1.reshape(128, 1)),
        "w2": _f16(W2),
        "b22": np.ascontiguousarray(np.concatenate([b2, b2]).reshape(128, 1)),
        "w3blk": _f16(w3blk),
        "b34": np.full((34, 1), np.float32(b3[0]), dtype=np.float32),
    }
    in_maps = []
    for c in range(N_CORES):
        sl = slice(c * nj, (c + 1) * nj)
        ut = _f16(np.concatenate([x_dag[sl], h_dag[sl]], axis=1).T)  # [80, nj]
        vt = _f16(h_glob[sl].T)  # [64, nj]
        in_maps.append({
            **shared, "ut": ut, "vt": vt,
            "spack": _build_spack(plans[c], num_exec),
        })

    trace = bool(int(os.environ.get("KERNEL_TRACE", "0")))
    if trace:
        _ensure_ntff_hook()
    res = run_bass_kernel_spmd(nc, in_maps, list(range(N_CORES)), trace=trace)
    last_results = res

    T = plan["T"]
    out = np.concatenate(
        [res.results[c]["out"].reshape(-1)[:T] for c in range(N_CORES)])
    assert out.shape[0] == int(total_actions)
    return out.astype(np.float32)


# revision 25
# speedup vs baseline: 1.3188x; 1.3188x over previous
"""Trainium2 Bass kernel for nn_ExecPolicyNetwork (ragged repeat + 3-layer MLP).

Math (reference):
    x_dag = x[ptr[:-1], :16][job_indices]                       # [N, 16]
    u = [x_dag | h_dag]  (80)   v = h_glob (64)
    dag_inputs[t] = [u[job(t)] | v[job(t)] | a_t]               # [T, 145]
    out = tanh(tanh(dag_inputs @ W1 + b1) @ W2 + b2) @ W3 + b3  # [T]

Kernel factorization: layer 1 is computed per-JOB (projUV = [u|v] @ W1[:144],
20000 rows instead of 1,010,000), then expanded to the ragged action dim by a
one-hot matmul on the PE that simultaneously adds the a_t * W1[144] rank-1 term:
    h1_pre[:, t] = [projUV_rows; w_a]^T @ [onehot(job)_cols; a_row]
Sharding: data-parallel over jobs, 8 contiguous slices of 2500 jobs (each
exactly 25 cycles of k=1..100 -> identical ragged structure on every core, so
one SPMD program serves all cores). All compute in fp16 (exact 0/1 one-hots,
fp32 PSUM accumulate); measured end-to-end relative error ~6e-4.

Per core, actions are processed in groups of 2048 (4 matmul tiles of 512).
PSUM (8 banks): exp ring [128,2048] banks 0-3, L2 ring [128,1024] banks 4-5,
L3+phase-1 in banks 6-7. Layer 2 runs as col-tiled concurrent pairs
(tile_position (0,0)/(0,64)); layer 3 as col-tiled concurrent pairs into one
bank (rows 0-1 / 32-33). Emission is software-pipelined: L3(g-1) fills the
PE's tanh1(g) wait window.
"""

import os
import numpy as np
from contextlib import ExitStack

from concourse import bacc, tile, mybir
from concourse.bass_utils import run_bass_kernel_spmd
from concourse._compat import with_exitstack

F32 = mybir.dt.float32
F16 = mybir.dt.float16
Tanh = mybir.ActivationFunctionType.Tanh
ADD = mybir.AluOpType.add

N_CORES = 8
NUM_DAG_FEATURES = 16
TILE = 512
TILES_PER_GROUP = 4
GROUP = TILE * TILES_PER_GROUP  # 2048

_cache = {}
last_results = None


def _f16(a):
    return np.ascontiguousarray(a, dtype=np.float16)


def _ensure_ntff_hook():
    """This image lacks antenv.axon_hooks; synthesize it so trace=True can
    capture NTFF profiles via /opt/axon/libaxon_pjrt.so."""
    import sys, types, ctypes, contextlib
    try:
        from antenv.axon_hooks import get_axon_ntff_profile_hook  # noqa: F401
        return
    except ImportError:
        pass
    so_path = "/opt/axon/libaxon_pjrt.so"
    if not os.path.exists(so_path):
        return
    lib = ctypes.CDLL(so_path)
    if not hasattr(lib, "axon_start_nrt_profile"):
        return
    lib.axon_start_nrt_profile.argtypes = [ctypes.POINTER(ctypes.c_int64), ctypes.c_size_t]
    lib.axon_start_nrt_profile.restype = ctypes.c_int64
    lib.axon_stop_nrt_profile.argtypes = [ctypes.c_char_p]
    lib.axon_stop_nrt_profile.restype = ctypes.c_int64

    @contextlib.contextmanager
    def _hook(output_dir, device_ids):
        import jax
        jax.devices()
        if device_ids:
            ids = (ctypes.c_int64 * len(device_ids))(*device_ids)
            rc = lib.axon_start_nrt_profile(ids, len(device_ids))
        else:
            rc = lib.axon_start_nrt_profile(None, 0)
        if rc != 0:
            raise RuntimeError(f"axon_start_nrt_profile rc={rc}")
        try:
            yield
        finally:
            n = lib.axon_stop_nrt_profile(str(output_dir).encode())
            print(f"ntff profile: {n} file(s) -> {output_dir}", file=sys.stderr)

    mod = types.ModuleType("antenv.axon_hooks")
    mod._hook = _hook
    mod.get_axon_ntff_profile_hook = lambda: _hook
    mod.set_axon_ntff_profile_hook = lambda h: setattr(mod, "_hook", h)
    import antenv
    sys.modules["antenv.axon_hooks"] = mod
    antenv.axon_hooks = mod


def _plan_core(k):
    """Static ragged plan for one core from its per-job action counts."""
    k = np.asarray(k, dtype=np.int64)
    nj = len(k)
    T = int(k.sum())
    n_groups = (T + GROUP - 1) // GROUP
    T_pad = n_groups * GROUP
    job_of_action = np.repeat(np.arange(nj), k)  # [T]
    start_of_job = np.concatenate([[0], np.cumsum(k)[:-1]])
    e_of_action = np.arange(T) - start_of_job[job_of_action]

    groups = []  # (jlo, jhi, srow_off, wide)
    srow_off = 0
    for g in range(n_groups):
        lo, hi = GROUP * g, min(GROUP * (g + 1), T)
        jlo = int(job_of_action[lo])
        jhi = int(job_of_action[hi - 1]) + 1
        assert jhi - jlo <= 67, "group spans too many jobs for one [68,128] lhsT"
        groups.append((jlo, jhi, srow_off))
        srow_off += (jhi - jlo) + 1
    return dict(
        T=T, T_pad=T_pad, n_groups=n_groups, groups=tuple(groups),
        srows=srow_off, job_of_action=job_of_action, e_of_action=e_of_action,
    )


def _build_spack(plan, num_exec):
    """Pack per-group [J_g+1, 2048] one-hot + a-row blocks into one array."""
    sp = np.zeros((plan["srows"], GROUP), dtype=np.float32)
    T = plan["T"]
    joa, eoa = plan["job_of_action"], plan["e_of_action"]
    a_vals = eoa.astype(np.float32) / np.float32(num_exec)
    for g, (jlo, jhi, off) in enumerate(plan["groups"]):
        lo, hi = GROUP * g, min(GROUP * (g + 1), T)
        cols = np.arange(lo, hi) - lo
        sp[off + (joa[lo:hi] - jlo), cols] = 1.0
        sp[off + (jhi - jlo), cols] = a_vals[lo:hi]
    return _f16(sp)


@with_exitstack
def _emit(ctx: ExitStack, tc: tile.TileContext, io, plan):
    nc = tc.nc
    n_groups = plan["n_groups"]
    groups = plan["groups"]
    NJ = io["ut"].shape[1]

    pool = ctx.enter_context(tc.tile_pool(name="consts", bufs=1))
    s_pool = ctx.enter_context(tc.tile_pool(name="s", bufs=3))
    gw_pool = ctx.enter_context(tc.tile_pool(name="gw", bufs=4))
    h1_pool = ctx.enter_context(tc.tile_pool(name="h1", bufs=2))
    h2_pool = ctx.enter_context(tc.tile_pool(name="h2", bufs=2))
    st_pool = ctx.enter_context(tc.tile_pool(name="st", bufs=3))

    t_ut = pool.tile([80, NJ], F16, tag="ut")
    nc.sync.dma_start(t_ut[:], io["ut"][:])
    t_vt = pool.tile([64, NJ], F16, tag="vt")
    nc.sync.dma_start(t_vt[:], io["vt"][:])
    t_w1a = pool.tile([80, 128], F16, tag="w1a")
    nc.sync.dma_start(t_w1a[:], io["w1a"][:])
    t_w1b = pool.tile([64, 128], F16, tag="w1b")
    nc.sync.dma_start(t_w1b[:], io["w1b"][:])
    t_b1 = pool.tile([128, 1], F32, tag="b1")
    nc.sync.dma_start(t_b1[:], io["b1"][:])
    t_w2 = pool.tile([128, 64], F16, tag="w2")
    nc.sync.dma_start(t_w2[:], io["w2"][:])
    t_b22 = pool.tile([128, 1], F32, tag="b22")
    nc.sync.dma_start(t_b22[:], io["b22"][:])
    t_w3 = pool.tile([128, 2], F16, tag="w3blk")
    nc.sync.dma_start(t_w3[:], io["w3blk"][:])
    t_b34 = pool.tile([34, 1], F32, tag="b34")
    nc.sync.dma_start(t_b34[:], io["b34"][:])

    # PSUM: exp ring banks 0-3, L2 ring banks 4-5, L3 + phase-1 banks 6-7.
    EXPR = nc.alloc_psum_tensor("EXPR", [128, GROUP], F32)
    L2R = nc.alloc_psum_tensor("L2R", [128, 1024], F32)
    R3 = [nc.alloc_psum_tensor(f"R3{i}", [128, 512], F32) for i in range(2)]

    s_tiles, gw_tiles, h1_tiles, h2_tiles = {}, {}, {}, {}

    def emit_s_dma(g):
        if g >= n_groups:
            return
        jlo, jhi, off = groups[g]
        rows = jhi - jlo + 1
        t = s_pool.tile([68, GROUP], F16, tag="s")
        nc.sync.dma_start(t[0:rows, :], io["spack"][off:off + rows, :])
        s_tiles[g] = t

    def emit_ph1(g):
        """projUV for group g's jobs -> gw tile (dual copy at bases 0 and 64
        for narrow groups, written by col-tiled matmuls — concurrent)."""
        if g >= n_groups:
            return
        jlo, jhi, off = groups[g]
        J = jhi - jlo
        c = 128 * (1 + (g // 2) % 3)
        pj = R3[g % 2].ap()[0:J, c:c + 128]
        nc.tensor.matmul(pj, t_ut[:, jlo:jhi], t_w1a[:], start=True, stop=False)
        nc.tensor.matmul(pj, t_vt[:, jlo:jhi], t_w1b[:], start=False, stop=True)
        t = gw_pool.tile([68, 128], F16, tag="gw")
        nc.vector.tensor_copy(t[0:J, :], pj)
        # w_a row lands on partition J — DVE cannot shift partitions, DMA can
        nc.gpsimd.dma_start(t[J:J + 1, :], io["wa"][:])
        gw_tiles[g] = t

    def emit_exp(g):
        """Expansion for group g: tile pairs run concurrently in disjoint PE
        row-groups (lhsT/rhs copies at partition bases 0 and 64)."""
        if g >= n_groups:
            return
        jlo, jhi, off = groups[g]
        rows = jhi - jlo + 1
        gw = gw_tiles.pop(g)
        s = s_tiles.pop(g)
        for t in range(TILES_PER_GROUP):
            nc.tensor.matmul(
                EXPR.ap()[:, TILE * t: TILE * (t + 1)],
                gw[0:rows, :],
                s[0:rows, TILE * t: TILE * (t + 1)],
                start=True, stop=True,
            )

    def emit_tanh1(g):
        """Two [128,1024] tanh ops so exp(g+1) can ring-reuse banks early."""
        if g >= n_groups:
            return
        h1 = h1_pool.tile([128, GROUP], F16, tag="h1")
        for h in range(2):
            nc.scalar.activation(h1[:, 1024 * h: 1024 * (h + 1)],
                                 EXPR.ap()[:, 1024 * h: 1024 * (h + 1)],
                                 Tanh, bias=t_b1[:, 0:1])
        h1_tiles[g] = h1

    def emit_l2(g):
        if g >= n_groups:
            return
        h1 = h1_tiles.pop(g)
        for p in range(2):
            nc.tensor.matmul(
                L2R.ap()[0:64, TILE * p: TILE * (p + 1)],
                t_w2[:], h1[:, 1024 * p: 1024 * p + 512],
                start=True, stop=True, tile_position=(0, 0),
            )
            nc.tensor.matmul(
                L2R.ap()[64:128, TILE * p: TILE * (p + 1)],
                t_w2[:], h1[:, 1024 * p + 512: 1024 * p + 1024],
                start=True, stop=True, tile_position=(0, 64),
            )

    def emit_tanh2(g):
        if g >= n_groups:
            return
        h2 = h2_pool.tile([128, 1024], F16, tag="h2")
        nc.scalar.activation(h2[:], L2R.ap()[:, 0:1024], Tanh, bias=t_b22[:, 0:1])
        h2_tiles[g] = h2

    def emit_l3_out(g):
        """Two col-tiled concurrent L3 matmuls into one R3 bank (rows 0-1 and
        32-33), one [34,512] DVE copy+bias, two out DMAs."""
        if g >= n_groups:
            return
        h2 = h2_tiles.pop(g)
        r3 = R3[g % 2].ap()
        nc.tensor.matmul(r3[0:2, 0:512], t_w3[:], h2[:, 0:512],
                         start=True, stop=True, tile_position=(0, 0))
        nc.tensor.matmul(r3[32:34, 0:512], t_w3[:], h2[:, 512:1024],
                         start=True, stop=True, tile_position=(0, 32))
        st = st_pool.tile([34, 512], F32, tag="st")
        nc.vector.tensor_scalar(st[:], r3[0:34, 0:512], t_b34[:, 0:1], None, ADD)
        nc.gpsimd.dma_start(io["out"][4 * g: 4 * g + 2, :], st[0:2, :])
        nc.gpsimd.dma_start(io["out"][4 * g + 2: 4 * g + 4, :], st[32:34, :])

    # ---- software-pipelined emission ----
    # PE order per iter: L3(g-1) fills the tanh1(g) wait, then L2(g),
    # ph1(g+2), exp(g+1). L3 is delayed one group (h2 pool holds it).
    emit_ph1(0)
    emit_ph1(1)
    emit_s_dma(0)
    emit_s_dma(1)
    emit_exp(0)
    for g in range(n_groups):
        emit_s_dma(g + 2)
        emit_tanh1(g)
        if g >= 1:
            emit_l3_out(g - 1)
        emit_l2(g)
        emit_ph1(g + 2)
        emit_exp(g + 1)
        emit_tanh2(g)
    emit_l3_out(n_groups - 1)


def _build(plan, nj):
    nc = bacc.Bacc(trn_type="TRN2", target_bir_lowering=False, debug=False)
    io = {
        "ut": nc.dram_tensor("ut", [80, nj], F16, kind="ExternalInput").ap(),
        "vt": nc.dram_tensor("vt", [64, nj], F16, kind="ExternalInput").ap(),
        "w1a": nc.dram_tensor("w1a", [80, 128], F16, kind="ExternalInput").ap(),
        "w1b": nc.dram_tensor("w1b", [64, 128], F16, kind="ExternalInput").ap(),
        "wa": nc.dram_tensor("wa", [1, 128], F16, kind="ExternalInput").ap(),
        "b1": nc.dram_tensor("b1", [128, 1], F32, kind="ExternalInput").ap(),
        "w2": nc.dram_tensor("w2", [128, 64], F16, kind="ExternalInput").ap(),
        "b22": nc.dram_tensor("b22", [128, 1], F32, kind="ExternalInput").ap(),
        "w3blk": nc.dram_tensor("w3blk", [128, 2], F16, kind="ExternalInput").ap(),
        "b34": nc.dram_tensor("b34", [34, 1], F32, kind="ExternalInput").ap(),
        "spack": nc.dram_tensor("spack", [plan["srows"], GROUP], F16, kind="ExternalInput").ap(),
        "out": nc.dram_tensor("out", [plan["T_pad"] // TILE, TILE], F32, kind="ExternalOutput").ap(),
    }
    with tile.TileContext(nc) as tc:
        _emit(tc, io, plan)
    nc.compile()
    return nc


def kernel(x, h_dag, h_glob, W1, b1, W2, b2, W3, b3,
           ptr, job_indices, exec_mask, num_exec_acts, total_actions):
    global last_results
    x = np.asarray(x, dtype=np.float32)
    h_dag = np.asarray(h_dag, dtype=np.float32)
    h_glob = np.asarray(h_glob, dtype=np.float32)
    W1 = np.asarray(W1, dtype=np.float32)
    b1 = np.asarray(b1, dtype=np.float32)
    W2 = np.asarray(W2, dtype=np.float32)
    b2 = np.asarray(b2, dtype=np.float32)
    W3 = np.asarray(W3, dtype=np.float32)
    b3 = np.asarray(b3, dtype=np.float32)
    ptr = np.asarray(ptr, dtype=np.int64)
    job_indices = np.asarray(job_indices, dtype=np.int64)
    k = np.asarray(num_exec_acts, dtype=np.int64)
    num_exec = np.asarray(exec_mask).shape[1]

    nj_total = len(job_indices)
    assert nj_total % N_CORES == 0
    nj = nj_total // N_CORES

    # per-job gathered features (host-side layout/gather only; no arithmetic)
    x_dag = x[ptr[:-1][job_indices], :NUM_DAG_FEATURES]  # [N, 16]

    # per-core plans must be identical (one SPMD program for all cores)
    plans = [_plan_core(k[c * nj:(c + 1) * nj]) for c in range(N_CORES)]
    key0 = (plans[0]["n_groups"], plans[0]["srows"], plans[0]["groups"])
    for p in plans[1:]:
        assert (p["n_groups"], p["srows"], p["groups"]) == key0, \
            "per-core ragged structures differ; SPMD single-program assumption violated"
    plan = plans[0]

    cache_key = (nj, key0)
    if cache_key not in _cache:
        _cache[cache_key] = _build(plan, nj)
    nc = _cache[cache_key]

    w3blk = np.zeros((128, 2), dtype=np.float32)
    w3blk[:64, 0] = W3[:, 0]
    w3blk[64:, 1] = W3[:, 0]
    shared = {
        "w1a": _f16(W1[:80]),
        "w1b": _f16(W1[80:144]),
        "wa": _f16(W1[144:145]),
        "b1": np.ascontiguousarray(b1.reshape(128, 1)),
        "w2": _f16(W2),
        "b22": np.ascontiguousarray(np.concatenate([b2, b2]).reshape(128, 1)),
        "w3blk": _f16(w3blk),
        "b34": np.full((34, 1), np.float32(b3[0]), dtype=np.float32),
    }
    in_maps = []
    for c in range(N_CORES):
        sl = slice(c * nj, (c + 1) * nj)
        ut = _f16(np.concatenate([x_dag[sl], h_dag[sl]], axis=1).T)  # [80, nj]
        vt = _f16(h_glob[sl].T)  # [64, nj]
        in_maps.append({
            **shared, "ut": ut, "vt": vt,
            "spack": _build_spack(plans[c], num_exec),
        })

    trace = bool(int(os.environ.get("KERNEL_TRACE", "0")))
    if trace:
        _ensure_ntff_hook()
    res = run_bass_kernel_spmd(nc, in_maps, list(range(N_CORES)), trace=trace)
    last_results = res

    T = plan["T"]
    out = np.concatenate(
        [res.results[c]["out"].reshape(-1)[:T] for c in range(N_CORES)])
    assert out.shape[0] == int(total_actions)
    return out.astype(np.float32)
